# revision 23
# baseline (speedup 1.0000x reference)
"""Trainium2 Bass kernel for DecoderWithAttention (show-attend-tell decoder).

Strategy (8 NeuronCores):
  - Batch-sharded recurrence: core c owns samples 8c..8c+7. Zero per-step
    collectives.
  - Attention-weight fusion: encW[b] = enc[b] @ W_awe.T is precomputed on
    device ([P,4H] per sample), so the per-step attention einsum
    sum_p alpha[b,p]*encW[b,p,:] directly yields the awe contribution to the
    layer-0 LSTM gates (awe itself is never materialized).
  - Fully transposed recurrence: gates live as [128, 16, 8] (feature-major,
    batch on the free dim), so LSTM state updates are 32-column vector ops,
    hidden states never need per-step transposes, and attention scores are
    computed feature-major (e^T via PE) so the softmax scatter is a set of
    partition-aligned on-chip copies instead of DMAs.
  - sigmoid(x) = (tanh(x/2)+1)/2: every activation (exp/tanh/relu) lives in
    the single `exp_and_others` table, so the activation table is loaded
    exactly once. h and c states are stored scaled by 2; the 0.5 factors are
    folded into weights host-side.
  - Output projection: one AllGather of per-step hidden states, then each
    core projects ALL 64*T rows against its 4000-column vocab slice of Wfc.
"""

import numpy as np
import ml_dtypes
from contextlib import ExitStack

import concourse.bass as bass
import concourse.bacc as bacc
import concourse.tile as tile
from concourse import mybir
from concourse.bass_utils import run_bass_kernel_spmd

BF16 = ml_dtypes.bfloat16
FP8 = ml_dtypes.float8_e4m3

B, PP, ENC, ATT, E, H, V = 64, 196, 2048, 512, 512, 512, 32000
NCORES = 8
BL = B // NCORES            # 8 samples per core
BP = BL * PP                # 1568 flattened (b,p) rows per core
KT_BP = (BP + 127) // 128   # 13 k-tiles over (b,p)
G = 4 * H                   # 2048 gate width
MG = G // 128               # 16 gate tiles
VS = V // NCORES            # 4000 vocab slice per core
bf16 = mybir.dt.bfloat16
f32 = mybir.dt.float32

Add = mybir.AluOpType.add
Mult = mybir.AluOpType.mult
Max = mybir.AluOpType.max
Tanh = mybir.ActivationFunctionType.Tanh
Exp = mybir.ActivationFunctionType.Exp
Relu = mybir.ActivationFunctionType.Relu

_PROG_CACHE = {}


def _bd_segments():
    """Rectangles of the block-diagonal alpha layout: (ktile j, r0, r1, b)."""
    segs = []
    for b in range(BL):
        lo, hi = b * PP, (b + 1) * PP
        f = lo
        while f < hi:
            j, r = divmod(f, 128)
            n = min(hi - f, 128 - r)
            segs.append((j, r, r + n, b))
            f += n
    return segs


def build_program(T, single_core=False):
    nc = bacc.Bacc()
    dt_in = {}

    def inp(name, shape, dtype=bf16):
        dt_in[name] = nc.declare_dram_parameter(name, list(shape), dtype, isOutput=False)
        return dt_in[name]

    TB = T * BL

    enc_dT = inp("enc_dt", [ENC, BP])            # per-core, d-major encoder
    embsT = inp("embst", [E + 1, TB])            # per-core, aug ones row
    wea = inp("wea", [ENC, ATT])
    wembt = inp("wembt", [E + 1, G])             # aug bias0 row (gate-reordered)
    whh0t = inp("whh0t", [H, G])                 # x0.5 (acts on 2h)
    wih1t = inp("wih1t", [H, G])                 # x0.5
    whh1t = inp("whh1t", [H, G])                 # x0.5
    wda = inp("wda", [H, ATT])                   # x0.5
    wfa = inp("wfa", [ATT, 1])
    wih2 = inp("wih2", [ENC, H])                 # x2 (init produces 2h)
    wic2 = inp("wic2", [ENC, H])                 # x2 (init produces 2c)
    wawet8 = inp("wawet8", [ENC, G], mybir.dt.float8e4)  # gate-reordered, x64
    wawetb = inp("wawetb", [ENC, G])             # gate-reordered, x64, bf16
    wfct = inp("wfct", [H, VS])                  # per-core vocab slice, x0.5
    bc_att = inp("bc_att", [1, ATT], f32)        # bea + bda (row)
    bias1t = inp("bias1t", [128, MG], f32)       # b_ih1+b_hh1 reordered, tiled
    bih2 = inp("bih2", [1, H], f32)
    bic2 = inp("bic2", [1, H], f32)
    bfc_row = inp("bfc_row", [1, VS])            # per-core bfc slice (bf16 row)
    ident8 = inp("ident8", [8, 8], f32)
    bdones = inp("bdones", [128, KT_BP * BL])    # block-diag ones mask

    M_ALL = NCORES * TB                          # rows in gathered projection
    pred = nc.declare_dram_parameter("pred", [M_ALL, VS], f32, isOutput=True)

    with tile.TileContext(nc) as tc, ExitStack() as ctx:
        # ---------------- persistent pool (lives whole kernel) ----------
        pw = ctx.enter_context(tc.tile_pool(name="pw", bufs=1))
        ctx2 = ctx.enter_context(ExitStack())
        pw_big = ctx2.enter_context(tc.tile_pool(name="pw_big", bufs=1))
        encw = pw_big.tile([128, KT_BP, G], bf16, tag="encw")
        att1t = pw_big.tile([128, 4, BP], bf16, tag="att1t")
        gembT = pw_big.tile([128, MG, TB], bf16, tag="gembT")
        h1t_all = pw.tile([128, 4, TB], bf16, tag="h1t_all")
        bd_pp = [pw.tile([128, KT_BP, BL], bf16, tag=f"bd{i}", name=f"bd{i}")
                 for i in range(2)]
        expT = pw.tile([128, KT_BP], bf16, tag="expT")
        bdones_s = pw.tile([128, KT_BP, BL], bf16, tag="bdones")
        bc_att_s = pw.tile([1, ATT], f32, tag="bc_att")
        bias1t_s = pw.tile([128, MG], f32, tag="bias1t")
        ones18 = pw.tile([1, 8], f32, tag="ones18")
        ones1x = pw.tile([1, 128], f32, tag="ones1x")
        id8 = pw.tile([8, 8], f32, tag="id8")
        nc.vector.memset(bd_pp[0], 0.0)
        nc.vector.memset(bd_pp[1], 0.0)
        nc.vector.memset(expT, 0.0)
        nc.vector.memset(ones18, 1.0)
        nc.vector.memset(ones1x, 1.0)
        nc.sync.dma_start(out=bdones_s, in_=bdones[:])
        nc.sync.dma_start(out=bc_att_s, in_=bc_att[:])
        nc.sync.dma_start(out=bias1t_s, in_=bias1t[:])
        nc.sync.dma_start(out=id8, in_=ident8[:])

        # state tiles (ping-pong via python refs); h/c stored scaled by 2
        ps_state = ctx2.enter_context(tc.tile_pool(name="state", bufs=1))
        h0t_pp = [ps_state.tile([128, 4, BL], bf16, tag=f"h0t_{i}", name=f"h0t_{i}") for i in range(2)]
        h1t_init = ps_state.tile([128, 4, BL], bf16, tag="h1t_init")
        sig0_pp = [ps_state.tile([128, 4, BL], f32, tag=f"s0_{i}", name=f"s0_{i}") for i in range(2)]
        sig1_pp = [ps_state.tile([128, 4, BL], f32, tag=f"s1_{i}", name=f"s1_{i}") for i in range(2)]

        # ---------------- precompute phase ------------------------------
        with tc.tile_pool(name="pre", bufs=1) as pre, \
             tc.tile_pool(name="pre2", bufs=3) as pre2, \
             tc.tile_pool(name="ppsum", bufs=1, space="PSUM") as ppsum, \
             tc.tile_pool(name="pipsum", bufs=1, space="PSUM") as pipsum:
            encdt_s = pre.tile([128, 16, BP], bf16, tag="encdt")
            for kt in range(16):
                nc.sync.dma_start(out=encdt_s[:, kt, :], in_=enc_dT[kt * 128:(kt + 1) * 128, :])

            # --- att1t = (enc @ Wea).T  (att-major) ---
            wbig = pre.tile([128, 16, ATT], bf16, tag="wbig")
            for kt in range(16):
                nc.sync.dma_start(out=wbig[:, kt, :], in_=wea[kt * 128:(kt + 1) * 128, :])
            HB = BP // 2
            for mt in range(4):
                for hf in range(2):
                    p_att1 = ppsum.tile([128, HB], f32, tag="pp2")
                    for c0 in range(0, HB, 512):
                        cw = min(512, HB - c0)
                        for kt in range(16):
                            nc.tensor.matmul(
                                p_att1[:, c0:c0 + cw],
                                wbig[:, kt, mt * 128:(mt + 1) * 128],
                                encdt_s[:, kt, hf * HB + c0:hf * HB + c0 + cw],
                                start=(kt == 0), stop=(kt == 15))
                    nc.vector.tensor_copy(att1t[:, mt, hf * HB:(hf + 1) * HB], p_att1)

            # --- mean over p (scaled), feature-major ---
            meant = pre.tile([128, 16, BL], bf16, tag="meant")
            meant_f = pre.tile([128, 16, BL], f32, tag="meantf")
            for kt in range(16):
                nc.vector.reduce_sum(
                    meant_f[:, kt, :],
                    encdt_s[:, kt, :].rearrange("p (b q) -> p b q", b=BL),
                    axis=mybir.AxisListType.X)
            nc.vector.tensor_scalar_mul(meant, meant_f, 1.0 / PP)

            # --- h0/c0 init, transposed [128, 4, 8]; values = 2h0 / 2c0 ---
            for (wsrc, bsrc, outs, odt) in (
                    (wih2, bih2, (h0t_pp[0], h1t_init), bf16),
                    (wic2, bic2, (sig0_pp[0], sig1_pp[0]), f32)):
                winit = pre.tile([128, 16, H], bf16, tag="wbig")
                for kt in range(16):
                    nc.sync.dma_start(out=winit[:, kt, :], in_=wsrc[kt * 128:(kt + 1) * 128, :])
                b_r = pre.tile([1, H], f32, tag="binit")
                nc.sync.dma_start(out=b_r, in_=bsrc[:])
                p_i = pipsum.tile([128, 4, BL], f32, tag="pi")
                for mh in range(4):
                    for kt in range(16):
                        nc.tensor.matmul(p_i[:, mh, :],
                                         winit[:, kt, mh * 128:(mh + 1) * 128],
                                         meant[:, kt, :],
                                         start=(kt == 0), stop=False)
                    nc.tensor.matmul(p_i[:, mh, :],
                                     b_r[0:1, mh * 128:(mh + 1) * 128], ones18,
                                     start=False, stop=True)
                nc.vector.tensor_copy(outs[0], p_i)
                nc.vector.tensor_copy(outs[1], outs[0])

            # --- gembT = (W_emb^T @ embsT) [128, MG, TB] (+bias0 ones row) ---
            wemb_s = pre.tile([128, 5, G], bf16, tag="wbig")
            for kt in range(4):
                nc.sync.dma_start(out=wemb_s[:, kt, :], in_=wembt[kt * 128:(kt + 1) * 128, :])
            nc.sync.dma_start(out=wemb_s[0:1, 4, :], in_=wembt[E:E + 1, :])
            embst_s = pre.tile([128, 5, TB], bf16, tag="embst")
            for kt in range(4):
                nc.sync.dma_start(out=embst_s[:, kt, :], in_=embsT[kt * 128:(kt + 1) * 128, :])
            nc.sync.dma_start(out=embst_s[0:1, 4, :], in_=embsT[E:E + 1, :])
            for mg in range(MG):
                p_ge = ppsum.tile([128, TB], f32, tag="pp2")
                for kt in range(4):
                    nc.tensor.matmul(p_ge, wemb_s[:, kt, mg * 128:(mg + 1) * 128],
                                     embst_s[:, kt, :], start=(kt == 0), stop=False)
                nc.tensor.matmul(p_ge, wemb_s[0:1, 4, mg * 128:(mg + 1) * 128],
                                 embst_s[0:1, 4, :], start=False, stop=True)
                nc.vector.tensor_copy(gembT[:, mg, :], p_ge)

            # --- encW = enc @ W_awe.T, (b,p)-major; fp8e4 DoubleRow ---
            # wawet8 is pre-scaled x64 host-side (fp8 subnormal range);
            # the 1/64 is folded back in the PSUM->SBUF copy.
            fp8 = mybir.dt.float8e4
            DR = mybir.MatmulPerfMode.DoubleRow
            encdt8 = pre.tile([128, 8, BP], fp8, tag="encdt8")
            for kt in range(8):
                nc.vector.tensor_copy(encdt8[:, kt, :], encdt_s[:, kt, :])
            NBLK = 256
            for nb in range(G // NBLK):
                p_e1 = ppsum.tile([128, 5, NBLK], f32, tag="pp1")
                p_e2 = ppsum.tile([128, 4, NBLK], f32, tag="pp2")
                p_e3 = ppsum.tile([128, 4, NBLK], f32, tag="pp3")
                wstrip = pre2.tile([128, 8, NBLK], fp8, tag="wstrip")
                for kt in range(8):
                    nc.sync.dma_start(
                        out=wstrip[:, kt, :],
                        in_=wawet8[kt * 128:(kt + 1) * 128, nb * NBLK:(nb + 1) * NBLK])
                # mt-outer / kt-inner: accumulation groups sharing a PSUM bank
                # must run sequentially
                wstripb = pre2.tile([128, 8, NBLK], bf16, tag="wstripb")
                for kt in range(8, 16):
                    nc.sync.dma_start(
                        out=wstripb[:, kt - 8, :],
                        in_=wawetb[kt * 128:(kt + 1) * 128, nb * NBLK:(nb + 1) * NBLK])
                for mt in range(KT_BP):
                    mw = min(128, BP - mt * 128)
                    if mt < 5:
                        tgt = p_e1[:mw, mt, :]
                    elif mt < 9:
                        tgt = p_e2[:mw, mt - 5, :]
                    else:
                        tgt = p_e3[:mw, mt - 9, :]
                    for kt in range(0, 8, 2):
                        nc.tensor.matmul(tgt, encdt8[:, kt:kt + 2, mt * 128:mt * 128 + mw],
                                         wstrip[:, kt:kt + 2, :], start=(kt == 0),
                                         stop=False, perf_mode=DR)
                    for kt in range(8, 16):
                        nc.tensor.matmul(tgt, encdt_s[:, kt, mt * 128:mt * 128 + mw],
                                         wstripb[:, kt - 8, :], start=False,
                                         stop=(kt == 15))
                for mt in range(KT_BP):
                    mw = min(128, BP - mt * 128)
                    if mt < 5:
                        psrc = p_e1[:mw, mt, :]
                    elif mt < 9:
                        psrc = p_e2[:mw, mt - 5, :]
                    else:
                        psrc = p_e3[:mw, mt - 9, :]
                    nc.vector.tensor_scalar_mul(encw[:mw, mt, nb * NBLK:(nb + 1) * NBLK],
                                                psrc, 1.0 / 64.0)

        # ---------------- recurrence weights (loaded after precompute) --
        pwts = ctx2.enter_context(tc.tile_pool(name="pwts", bufs=1))
        w0 = pwts.tile([128, 4, G], bf16, tag="w0")
        w1 = pwts.tile([128, 4, G], bf16, tag="w1")
        w2 = pwts.tile([128, 4, G], bf16, tag="w2")
        wda_s = pwts.tile([128, 4, ATT], bf16, tag="wda")
        wfa_s = pwts.tile([128, 4, 1], bf16, tag="wfa")
        for kt in range(4):
            nc.sync.dma_start(out=w0[:, kt, :], in_=whh0t[kt * 128:(kt + 1) * 128, :])
        for kt in range(4):
            nc.sync.dma_start(out=w1[:, kt, :], in_=wih1t[kt * 128:(kt + 1) * 128, :])
        for kt in range(4):
            nc.sync.dma_start(out=w2[:, kt, :], in_=whh1t[kt * 128:(kt + 1) * 128, :])
        for kt in range(4):
            nc.sync.dma_start(out=wda_s[:, kt, :], in_=wda[kt * 128:(kt + 1) * 128, :])
        for kt in range(4):
            nc.sync.dma_start(out=wfa_s[:, kt, :], in_=wfa[kt * 128:(kt + 1) * 128, :])

        # ---------------- recurrence ------------------------------------
        with tc.tile_pool(name="rec", bufs=1) as rec, \
             tc.tile_pool(name="rpsA", bufs=1, space="PSUM") as rpsA, \
             tc.tile_pool(name="rpsB", bufs=1, space="PSUM") as rpsB:
            for t in range(T):
                cur, nxt = t % 2, (t + 1) % 2
                h0T = h0t_pp[cur]
                h1T = h1t_init if t == 0 else h1t_all[:, :, (t - 1) * BL: t * BL]
                bd = bd_pp[t % 2]

                # att2T [128, 4, 8] (att-major) with bea+bda folded in via
                # a rank-1 bias matmul
                p_a2 = rpsA.tile([128, 4, BL], f32, tag="pa2")
                for mta in range(4):
                    for kth in range(4):
                        nc.tensor.matmul(p_a2[:, mta, :],
                                         wda_s[:, kth, mta * 128:(mta + 1) * 128],
                                         h1T[:, kth, :], start=(kth == 0), stop=False)
                    nc.tensor.matmul(p_a2[:, mta, :],
                                     bc_att_s[0:1, mta * 128:(mta + 1) * 128], ones18,
                                     start=False, stop=True)

                # relu(att1 + att2) in a single fused pass: att2 is a
                # per-partition scalar for each (att-tile, sample) slice.
                # Split across DVE / Act / Pool by their per-call rates.
                att2s = rec.tile([128, 4, BL], f32, tag="att2s")
                nc.vector.tensor_copy(att2s, p_a2)
                relu_s = rec.tile([128, 4, BP], bf16, tag="relu")
                relu_eng = (["D"] * 14 + ["A"] * 8 + ["P"] * 10)
                relu_eng = [relu_eng[(i * 11) % 32] for i in range(32)]
                for mta in range(4):
                    for b in range(BL):
                        sl = slice(b * PP, (b + 1) * PP)
                        eng = relu_eng[mta * BL + b]
                        if eng == "A":
                            nc.scalar.activation(
                                out=relu_s[:, mta, sl], in_=att1t[:, mta, sl],
                                func=Relu, bias=att2s[:, mta, b:b + 1])
                        else:
                            e = nc.vector if eng == "D" else nc.gpsimd
                            e.tensor_scalar(out=relu_s[:, mta, sl],
                                            in0=att1t[:, mta, sl],
                                            scalar1=att2s[:, mta, b:b + 1],
                                            scalar2=0.0, op0=Add, op1=Max)

                # eT [128, 13] = relu^T @ wfa (feature-major scores)
                p_eT = rpsA.tile([128, KT_BP], f32, tag="peT")
                for j in range(KT_BP):
                    mw = min(128, BP - j * 128)
                    for kta in range(4):
                        nc.tensor.matmul(p_eT[:mw, j:j + 1],
                                         relu_s[:, kta, j * 128:j * 128 + mw],
                                         wfa_s[:, kta, :],
                                         start=(kta == 0), stop=(kta == 3))

                # exp (only valid rows; expT pre-zeroed once)
                nc.scalar.activation(out=expT[:, 0:KT_BP - 1], in_=p_eT[:, 0:KT_BP - 1], func=Exp)
                lastw = BP - (KT_BP - 1) * 128
                nc.scalar.activation(out=expT[0:lastw, KT_BP - 1:KT_BP],
                                     in_=p_eT[0:lastw, KT_BP - 1:KT_BP], func=Exp)

                # Z[b] = sum of sample-b scores via block-diag-ones matmul,
                # then 1/Z broadcast to all partitions via transpose+rank-1 mm
                p_z = rpsB.tile([BL, 1], f32, tag="pz")
                for j in range(KT_BP):
                    nc.tensor.matmul(p_z, bdones_s[:, j, :], expT[:, j:j + 1],
                                     start=(j == 0), stop=(j == KT_BP - 1))
                rinv_sb = rec.tile([BL, 1], f32, tag="rinv")
                nc.vector.reciprocal(rinv_sb, p_z)
                p_rt = rpsB.tile([1, BL], f32, tag="prt")
                nc.tensor.transpose(p_rt, rinv_sb, id8)
                rt_sb = rec.tile([1, BL], f32, tag="rt")
                nc.vector.tensor_copy(rt_sb, p_rt)
                p_rb = rpsB.tile([128, BL], f32, tag="prb")
                nc.tensor.matmul(p_rb, ones1x, rt_sb, start=True, stop=True)
                rb_s = rec.tile([128, BL], f32, tag="rbs")
                nc.vector.tensor_copy(rb_s, p_rb)

                # alpha block-diagonal: bd = (bdones * expT[r, j]) * rinv[b].
                # The mask*exp product (Pool) overlaps the 1/Z chain.
                bdr = rec.tile([128, KT_BP, BL], f32, tag="bdr")
                ev = expT[:, :]
                e_bcast = bass.AP(tensor=ev.tensor, offset=ev.offset,
                                  ap=[ev.ap[0], ev.ap[1], [0, BL]])
                nc.gpsimd.tensor_tensor(out=bdr, in0=bdones_s, in1=e_bcast, op=Mult)
                rbv = rb_s[:, :]
                rb_bcast = bass.AP(tensor=rbv.tensor, offset=rbv.offset,
                                   ap=[rbv.ap[0], [0, KT_BP], rbv.ap[1]])
                nc.vector.tensor_tensor(out=bd, in0=bdr, in1=rb_bcast, op=Mult)

                # gates0^T [128, 16, 8] = W_hh0@h0 + alpha-einsum(encW)
                p_g0 = rpsA.tile([128, MG, BL], f32, tag="pg0")
                for mg in range(MG):
                    for kth in range(4):
                        nc.tensor.matmul(p_g0[:, mg, :],
                                         w0[:, kth, mg * 128:(mg + 1) * 128],
                                         h0T[:, kth, :], start=(kth == 0), stop=False)
                    for j in range(KT_BP):
                        mw = min(128, BP - j * 128)
                        nc.tensor.matmul(p_g0[:, mg, :],
                                         encw[:mw, j, mg * 128:(mg + 1) * 128],
                                         bd[:mw, j, :],
                                         start=False, stop=(j == KT_BP - 1))
                g0s = rec.tile([128, MG, BL], f32, tag="g0s")
                nc.vector.tensor_tensor(out=g0s, in0=p_g0,
                                        in1=gembT[:, :, t * BL:(t + 1) * BL], op=Add)

                # LSTM cell 0 (tanh-only; states scaled by 2)
                tifo0 = rec.tile([128, 12, BL], bf16, tag="tifo0")
                tg0 = rec.tile([128, 4, BL], bf16, tag="tg0")
                nc.scalar.activation(out=tifo0, in_=g0s[:, 0:12, :], func=Tanh, scale=0.5)
                nc.scalar.activation(out=tg0, in_=g0s[:, 12:16, :], func=Tanh)
                a2c = rec.tile([128, 4, BL], f32, tag="a2c0")
                btc = rec.tile([128, 4, BL], f32, tag="btc0")
                nc.vector.scalar_tensor_tensor(out=a2c, in0=tifo0[:, 4:8, :], scalar=1.0,
                                               in1=sig0_pp[cur], op0=Add, op1=Mult)
                nc.vector.scalar_tensor_tensor(out=btc, in0=tifo0[:, 0:4, :], scalar=1.0,
                                               in1=tg0, op0=Add, op1=Mult)
                nc.vector.scalar_tensor_tensor(out=sig0_pp[nxt], in0=a2c, scalar=0.5,
                                               in1=btc, op0=Mult, op1=Add)
                thc0 = rec.tile([128, 4, BL], bf16, tag="thc0")
                nc.scalar.activation(out=thc0, in_=sig0_pp[nxt], func=Tanh, scale=0.5)
                nc.vector.scalar_tensor_tensor(out=h0t_pp[nxt], in0=tifo0[:, 8:12, :],
                                               scalar=1.0, in1=thc0, op0=Add, op1=Mult)

                # gates1^T = W_ih1@h0n + W_hh1@h1
                p_g1 = rpsB.tile([128, MG, BL], f32, tag="pg1")
                for mg in range(MG):
                    for kth in range(4):
                        nc.tensor.matmul(p_g1[:, mg, :],
                                         w2[:, kth, mg * 128:(mg + 1) * 128],
                                         h1T[:, kth, :], start=(kth == 0), stop=False)
                    for kth in range(4):
                        nc.tensor.matmul(p_g1[:, mg, :],
                                         w1[:, kth, mg * 128:(mg + 1) * 128],
                                         h0t_pp[nxt][:, kth, :],
                                         start=False, stop=(kth == 3))
                g1s = rec.tile([128, MG, BL], f32, tag="g1s")
                b1 = bias1t_s[:, :]
                b1_bcast = bass.AP(tensor=b1.tensor, offset=b1.offset,
                                   ap=[b1.ap[0], b1.ap[1], [0, BL]])
                nc.vector.tensor_tensor(out=g1s, in0=p_g1, in1=b1_bcast, op=Add)

                # LSTM cell 1
                tifo1 = rec.tile([128, 12, BL], bf16, tag="tifo1")
                tg1 = rec.tile([128, 4, BL], bf16, tag="tg1")
                nc.scalar.activation(out=tifo1, in_=g1s[:, 0:12, :], func=Tanh, scale=0.5)
                nc.scalar.activation(out=tg1, in_=g1s[:, 12:16, :], func=Tanh)
                a2c1 = rec.tile([128, 4, BL], f32, tag="a2c1")
                btc1 = rec.tile([128, 4, BL], f32, tag="btc1")
                nc.vector.scalar_tensor_tensor(out=a2c1, in0=tifo1[:, 4:8, :], scalar=1.0,
                                               in1=sig1_pp[cur], op0=Add, op1=Mult)
                nc.vector.scalar_tensor_tensor(out=btc1, in0=tifo1[:, 0:4, :], scalar=1.0,
                                               in1=tg1, op0=Add, op1=Mult)
                nc.vector.scalar_tensor_tensor(out=sig1_pp[nxt], in0=a2c1, scalar=0.5,
                                               in1=btc1, op0=Mult, op1=Add)
                thc1 = rec.tile([128, 4, BL], bf16, tag="thc1")
                nc.scalar.activation(out=thc1, in_=sig1_pp[nxt], func=Tanh, scale=0.5)
                nc.vector.scalar_tensor_tensor(out=h1t_all[:, :, t * BL:(t + 1) * BL],
                                               in0=tifo1[:, 8:12, :], scalar=1.0,
                                               in1=thc1, op0=Add, op1=Mult)

        ctx2.close()

        # ---------------- AllGather hidden states -----------------------
        with tc.tile_pool(name="dram", bufs=1, space="DRAM") as dpool:
            ag_in = dpool.tile([4, 128, TB], bf16)
            ag_out = dpool.tile([NCORES, 4, 128, TB], bf16,
                                **({} if single_core else {"addr_space": "Shared"}))
            nc.sync.dma_start(out=ag_in.rearrange("k p m -> p k m"), in_=h1t_all)
            if single_core:
                for r in range(NCORES):
                    nc.sync.dma_start(out=ag_out[r], in_=ag_in[:])
            else:
                nc.gpsimd.collective_compute(
                    "AllGather", mybir.AluOpType.bypass,
                    replica_groups=[list(range(NCORES))],
                    ins=[ag_in.opt()], outs=[ag_out.opt()])

            # ---------------- vocab-sharded projection ------------------
            with tc.tile_pool(name="proj", bufs=1) as proj, \
                 tc.tile_pool(name="proj2", bufs=2) as proj2, \
                 tc.tile_pool(name="jpsum", bufs=4, space="PSUM") as jpsum:
                hall = proj.tile([128, 4, M_ALL], bf16, tag="hall")
                for r in range(NCORES):
                    for kt in range(4):
                        nc.sync.dma_start(out=hall[:, kt, r * TB:(r + 1) * TB],
                                          in_=ag_out[r, kt])
                hall_ones = proj.tile([1, M_ALL], bf16, tag="hones")
                nc.vector.memset(hall_ones, 1.0)
                wfc_s = proj.tile([128, 4, VS], bf16, tag="wfc")
                for kt in range(4):
                    nc.sync.dma_start(out=wfc_s[:, kt, :], in_=wfct[kt * 128:(kt + 1) * 128, :])
                bfc_s = proj.tile([1, VS], bf16, tag="bfcs")
                nc.sync.dma_start(out=bfc_s, in_=bfc_row[:])
                NCH = 500
                NMT = M_ALL // 128
                dma_eng = [nc.sync, nc.scalar]
                for nch in range(VS // NCH):
                    o_big = proj2.tile([128, NMT, NCH], f32, tag="obig")
                    for mt in range(NMT):
                        p_p = jpsum.tile([128, NCH], f32, tag="pj")
                        for kt in range(4):
                            nc.tensor.matmul(p_p, hall[:, kt, mt * 128:(mt + 1) * 128],
                                             wfc_s[:, kt, nch * NCH:(nch + 1) * NCH],
                                             start=(kt == 0), stop=False)
                        nc.tensor.matmul(p_p, hall_ones[0:1, mt * 128:(mt + 1) * 128],
                                         bfc_s[0:1, nch * NCH:(nch + 1) * NCH],
                                         start=False, stop=True)
                        if mt % 2 == 0:
                            nc.vector.tensor_copy(o_big[:, mt, :], p_p)
                        else:
                            nc.scalar.copy(o_big[:, mt, :], p_p)
                    dma_eng[nch % 2].dma_start(
                        out=pred[:, nch * NCH:(nch + 1) * NCH].rearrange(
                            "(m p) c -> p m c", p=128),
                        in_=o_big)
    nc.compile()
    return nc


def kernel(**inputs):
    T = int(inputs["lengths"])
    enc = np.asarray(inputs["encoder_out"], np.float32)
    captions = np.asarray(inputs["captions"])
    emb = np.asarray(inputs["emb"], np.float32)

    # gate reorder: pytorch [i, f, g, o] -> kernel [i, f, o, g]
    perm = np.concatenate([np.arange(0, H), np.arange(H, 2 * H),
                           np.arange(3 * H, 4 * H), np.arange(2 * H, 3 * H)])
    W_ih0 = np.asarray(inputs["W_ih0"], np.float32)[perm]
    W_hh0 = np.asarray(inputs["W_hh0"], np.float32)[perm]
    W_ih1 = np.asarray(inputs["W_ih1"], np.float32)[perm]
    W_hh1 = np.asarray(inputs["W_hh1"], np.float32)[perm]
    bias0 = (np.asarray(inputs["b_ih0"], np.float32) + np.asarray(inputs["b_hh0"], np.float32))[perm]
    bias1 = (np.asarray(inputs["b_ih1"], np.float32) + np.asarray(inputs["b_hh1"], np.float32))[perm]
    bc_att = np.asarray(inputs["bea"], np.float32) + np.asarray(inputs["bda"], np.float32)

    bdones = np.zeros((128, KT_BP, BL), np.float32)
    for (j, r0, r1, b) in _bd_segments():
        bdones[r0:r1, j, b] = 1.0

    shared = {
        "wea": np.ascontiguousarray(inputs["Wea"]).astype(BF16),
        "wembt": np.concatenate([W_ih0[:, :E].T, bias0[None, :]], 0).astype(BF16),
        "whh0t": np.ascontiguousarray(W_hh0.T * 0.5).astype(BF16),
        "wih1t": np.ascontiguousarray(W_ih1.T * 0.5).astype(BF16),
        "whh1t": np.ascontiguousarray(W_hh1.T * 0.5).astype(BF16),
        "wda": (np.ascontiguousarray(inputs["Wda"]) * 0.5).astype(BF16),
        "wfa": np.ascontiguousarray(inputs["Wfa"]).astype(BF16),
        "wih2": (np.ascontiguousarray(inputs["Wih"]) * 2.0).astype(BF16),
        "wic2": (np.ascontiguousarray(inputs["Wic"]) * 2.0).astype(BF16),
        "wawet8": np.ascontiguousarray(W_ih0[:, E:].T * 64.0).astype(FP8),
        "wawetb": np.ascontiguousarray(W_ih0[:, E:].T * 64.0).astype(BF16),
        "bc_att": np.ascontiguousarray(bc_att[None, :]),
        "bias1t": np.ascontiguousarray(bias1.reshape(4 * H // 128, 128).T.astype(np.float32)),
        "bih2": np.ascontiguousarray(np.asarray(inputs["bih"], np.float32)[None, :] * 2.0),
        "bic2": np.ascontiguousarray(np.asarray(inputs["bic"], np.float32)[None, :] * 2.0),
        "ident8": np.eye(8, dtype=np.float32),
        "bdones": np.ascontiguousarray(bdones.reshape(128, KT_BP * BL)).astype(BF16),
    }
    Wfc = np.asarray(inputs["Wfc"], np.float32)
    bfc = np.asarray(inputs["bfc"], np.float32)
    embs = emb[np.asarray(captions, np.int64)]        # [B, T_cap, E] host gather

    in_maps = []
    for c in range(NCORES):
        enc_c = enc[c * BL:(c + 1) * BL]                       # [BL, P, ENC]
        enc_dT = np.ascontiguousarray(enc_c.reshape(BP, ENC).T).astype(BF16)
        e_c = embs[c * BL:(c + 1) * BL, :T]                    # [BL, T, E]
        embsT = np.ascontiguousarray(e_c.transpose(2, 1, 0).reshape(E, T * BL))
        embsT = np.concatenate([embsT, np.ones((1, T * BL), np.float32)], 0).astype(BF16)
        m = dict(shared)
        m["enc_dt"] = enc_dT
        m["embst"] = embsT
        m["wfct"] = np.ascontiguousarray(Wfc[:, c * VS:(c + 1) * VS] * 0.5).astype(BF16)
        m["bfc_row"] = np.ascontiguousarray(bfc[None, c * VS:(c + 1) * VS]).astype(BF16)
        in_maps.append(m)

    if T not in _PROG_CACHE:
        _PROG_CACHE[T] = build_program(T)
    nc = _PROG_CACHE[T]

    res = run_bass_kernel_spmd(nc, in_maps, list(range(NCORES)))
    globals()["LAST_RESULT"] = res
    outs = res.results

    # per-core pred: [NCORES*T*BL, VS]; rows = (src_rank, t, b_local)
    parts = [outs[c]["pred"].reshape(NCORES, T, BL, VS) for c in range(NCORES)]
    full = np.concatenate(parts, axis=-1)             # [NCORES, T, BL, V]
    return np.ascontiguousarray(full.transpose(0, 2, 1, 3).reshape(B, T, V)).astype(np.float32)


    import jax
    from jax.experimental.shard_map import shard_map
    from jax.sharding import Mesh, PartitionSpec
    from concourse import bass2jax

    if T not in _PROG_CACHE:
        _PROG_CACHE[T] = build_program(T)
    nc = _PROG_CACHE[T]
    bass2jax.install_neuronx_cc_hook()

    partition_name = nc.partition_id_tensor.name if nc.partition_id_tensor else None
    in_names, out_names, out_avals = [], [], []
    for alloc in nc.m.functions[0].allocations:
        if not isinstance(alloc, mybir.MemoryLocationSet):
            continue
        name = alloc.memorylocations[0].name
        if alloc.kind == "ExternalInput":
            if name != partition_name:
                in_names.append(name)
        elif alloc.kind == "ExternalOutput":
            out_names.append(name)
            out_avals.append(jax.core.ShapedArray(
                tuple(alloc.tensor_shape), mybir.dt.np(alloc.dtype)))
    n_params, n_outs = len(in_names), len(out_avals)
    all_names = in_names + out_names + ([partition_name] if partition_name else [])
    donate = tuple(range(n_params, n_params + n_outs))

    def _body(*args):
        operands = list(args)
        if partition_name is not None:
            operands.append(bass2jax.partition_id_tensor())
        outs = bass2jax._bass_exec_p.bind(
            *operands, out_avals=tuple(out_avals), in_names=tuple(all_names),
            out_names=tuple(out_names), lowering_input_output_aliases=(),
            sim_require_finite=True, sim_require_nnan=True, nc=nc)
        return tuple(outs)

    devices = jax.devices()[:NCORES]
    mesh = Mesh(np.asarray(devices), ("core",))
    fn = jax.jit(
        shard_map(_body, mesh=mesh,
                  in_specs=(PartitionSpec("core"),) * (n_params + n_outs),
                  out_specs=(PartitionSpec("core"),) * n_outs, check_rep=False),
        donate_argnums=donate, keep_unused=True)
    out_shapes = [(tuple(a.shape), a.dtype) for a in out_avals]
    _RUNNER_CACHE[T] = (fn, in_names, out_shapes)
    return _RUNNER_CACHE[T]
